# revision 17
# baseline (speedup 1.0000x reference)
"""VQ codebook (DiscreteLayer) Trainium2 kernel.

kernel(**inputs) takes FULL inputs
  x [64, 256, 4096] f32, embed [256, 32] f32, cluster_number [32] f32,
  embed_avg [256, 32] f32, training scalar
and returns (quantize [64,256,4096] f32, embed_out [256,32] f32, loss f32),
matching the jax reference semantics.

Sharded data-parallel over batches across 8 NeuronCores (8 batches/core).

Per-core dataflow (v3, measured-cost driven):
  - x streamed in quarter-batch tiles [128, 1024] (4KB-contiguous DMA runs)
  - GPSIMD splits x into fp16 hi/lo pairs (fh, fl) with exact residual
  - scores -2*f.e computed as fh*ph + fh*pl + fl*ph (fp16 matmuls, all
    products exact, psum fp32 accumulate -> fp32-matmul-class accuracy,
    but 1-cycle/row streams and cheap single-pass LDWEIGHTS)
  - x tile transposed on PE (fp32) -> token-major xts (fp16) for the
    embed_sum contraction; ||f||^2 per token reduced from the transposed
    psum on DVE (tensor_tensor_reduce) and ACT (Square+accum), split
  - dist = (-2f.e + ||f||^2) + ||e||^2 in the reference op order (fp32),
    argmin via reduce_min + is_equal one-hot (fp16)
  - quantize = embT(fp16) @ onehot via fp16 matmuls; embed_sum/counts as
    one long psum accumulation with the one-hot stationary and the
    token-major x (+ ones column for counts) streaming
Host: EMA epilogue on [32]/[256,32] + loss scalar + gather.
"""

import numpy as np

B, D, L, K = 64, 256, 4096, 32
NCORES = 8
BPC = B // NCORES          # batches per core
TT = 256                   # tokens per tile
QT = 1024                  # tokens per quarter-batch DMA tile
TPQ = QT // TT             # tiles per quarter
NQ = L // QT               # quarters per batch
NTILES = BPC * (L // TT)   # 128 tiles per core
DECAY = 0.99
EPS = 1e-5

_cached = {}


def _build_program():
    import concourse.bacc as bacc
    import concourse.mybir as mybir
    from concourse.tile import TileContext

    f32 = mybir.dt.float32
    f32r = mybir.dt.float32r
    f16 = mybir.dt.float16
    Alu = mybir.AluOpType
    Act = mybir.ActivationFunctionType
    X = mybir.AxisListType.X

    nc = bacc.Bacc("TRN2", target_bir_lowering=False, debug=False,
                   num_devices=NCORES)

    xd = nc.dram_tensor("x", [BPC, D, L], f32, kind="ExternalInput").ap()
    ph_d = nc.dram_tensor("ph", [D, K], f16, kind="ExternalInput").ap()
    phpl_d = nc.dram_tensor("phpl", [D, 2 * K], f16, kind="ExternalInput").ap()
    nek_d = nc.dram_tensor("nek", [128, 2 * K], f32, kind="ExternalInput").ap()
    embt_d = nc.dram_tensor("embt", [K, D], f16, kind="ExternalInput").ap()
    id32_d = nc.dram_tensor("id32", [128, 128], f32, kind="ExternalInput").ap()
    id32r_d = nc.dram_tensor("id32r", [128, 128], f32r, kind="ExternalInput").ap()
    ones_d = nc.dram_tensor("ones", [128, 2], f32r, kind="ExternalInput").ap()

    quant_d = nc.dram_tensor("quant", [BPC, D, L], f32, kind="ExternalOutput").ap()
    est_d = nc.dram_tensor("est", [K, D + 2], f32, kind="ExternalOutput").ap()
    loss_d = nc.dram_tensor("lo", [128, 1], f32, kind="ExternalOutput").ap()

    with TileContext(nc) as tc:
        with tc.tile_pool(name="const", bufs=1) as constp, \
             tc.tile_pool(name="xin", bufs=4) as xpool, \
             tc.tile_pool(name="fhl", bufs=4) as fpool, \
             tc.tile_pool(name="qb", bufs=4) as qbpool, \
             tc.tile_pool(name="xts", bufs=3) as xtspool, \
             tc.tile_pool(name="dist", bufs=3) as distpool, \
             tc.tile_pool(name="oht", bufs=3) as ohtpool, \
             tc.tile_pool(name="ohs", bufs=3) as ohspool, \
             tc.tile_pool(name="nf", bufs=4) as nfpool, \
             tc.tile_pool(name="scr", bufs=2) as scrpool, \
             tc.tile_pool(name="acc", bufs=1) as accpool, \
             tc.tile_pool(name="ps_xt", bufs=2, space="PSUM") as ps_xt, \
             tc.tile_pool(name="ps_sc", bufs=2, space="PSUM") as ps_sc, \
             tc.tile_pool(name="ps_oh", bufs=1, space="PSUM") as ps_oh, \
             tc.tile_pool(name="ps_q", bufs=2, space="PSUM") as ps_q, \
             tc.tile_pool(name="ps_es", bufs=1, space="PSUM") as ps_es:

            ph0 = constp.tile([128, K], f16)
            nc.sync.dma_start(out=ph0[:], in_=ph_d[0:128, :])
            ph1 = constp.tile([128, K], f16)
            nc.sync.dma_start(out=ph1[:], in_=ph_d[128:256, :])
            phpl0 = constp.tile([128, 2 * K], f16)
            nc.sync.dma_start(out=phpl0[:], in_=phpl_d[0:128, :])
            phpl1 = constp.tile([128, 2 * K], f16)
            nc.sync.dma_start(out=phpl1[:], in_=phpl_d[128:256, :])
            nek = constp.tile([128, 2 * K], f32)
            nc.sync.dma_start(out=nek[:], in_=nek_d[:])
            embt = constp.tile([K, D], f16)
            nc.sync.dma_start(out=embt[:], in_=embt_d[:])
            id32 = constp.tile([128, 128], f32)
            nc.sync.dma_start(out=id32[:], in_=id32_d[:])
            id32r = constp.tile([128, 128], f32r)
            nc.sync.dma_start(out=id32r[:], in_=id32r_d[:])
            ones = constp.tile([128, 2], f32r)
            nc.sync.dma_start(out=ones[:], in_=ones_d[:])

            lossbuf = accpool.tile([128, 2 * NTILES], f32)
            es_ps = ps_es.tile([K, D + 2], f32)

            def prep_quarter(b, qi):
                """DMA in one quarter and build its fp16 hi/lo split."""
                Q0 = qi * QT
                x0q = xpool.tile([128, QT], f32, tag="x")
                nc.sync.dma_start(out=x0q[:], in_=xd[b, 0:128, Q0:Q0 + QT])
                x1q = xpool.tile([128, QT], f32, tag="x")
                nc.sync.dma_start(out=x1q[:], in_=xd[b, 128:256, Q0:Q0 + QT])
                # casts on ACT, subtracts on GPSIMD (idle; DVE/ACT are busy)
                fh0 = fpool.tile([128, QT], f16, tag="fh")
                nc.scalar.copy(out=fh0[:], in_=x0q[:])
                fl0 = fpool.tile([128, QT], f16, tag="fl")
                nc.gpsimd.tensor_tensor(fl0[:], x0q[:], fh0[:], op=Alu.subtract)
                fh1 = fpool.tile([128, QT], f16, tag="fh")
                nc.scalar.copy(out=fh1[:], in_=x1q[:])
                fl1 = fpool.tile([128, QT], f16, tag="fl")
                nc.gpsimd.tensor_tensor(fl1[:], x1q[:], fh1[:], op=Alu.subtract)
                return x0q, x1q, fh0, fl0, fh1, fl1

            quarters = [(b, qi) for b in range(BPC) for qi in range(NQ)]
            pending = prep_quarter(*quarters[0])
            t = 0
            for bq in range(len(quarters)):
                b, qi = quarters[bq]
                Q0 = qi * QT
                x0q, x1q, fh0, fl0, fh1, fl1 = pending
                if bq + 1 < len(quarters):
                    pending = prep_quarter(*quarters[bq + 1])

                if True:
                    qb0 = qbpool.tile([128, QT], f32, tag="qb")
                    qb1 = qbpool.tile([128, QT], f32, tag="qb")

                    for i in range(TPQ):
                        to = i * TT          # token offset within quarter
                        first, last = (t == 0), (t == NTILES - 1)
                        s0, s1 = slice(to, to + 128), slice(to + 128, to + 256)

                        # T1: transpose x tile -> token-major (one psum bank)
                        xtp = ps_xt.tile([128, 512], f32)
                        nc.tensor.matmul(xtp[:, 0:128], x0q[:, s0], id32[:],
                                         is_transpose=True, start=True, stop=False)
                        nc.tensor.matmul(xtp[:, 128:256], x1q[:, s0], id32[:],
                                         is_transpose=True, start=False, stop=False)
                        nc.tensor.matmul(xtp[:, 256:384], x0q[:, s1], id32[:],
                                         is_transpose=True, start=False, stop=False)
                        nc.tensor.matmul(xtp[:, 384:512], x1q[:, s1], id32[:],
                                         is_transpose=True, start=False, stop=True)

                        # xts: token-major x as f32r (exact fp32 bits;
                        # f32r streams at 1 cyc/row in the M3 matmuls).
                        # Draining xtp via just these two copies releases the
                        # transpose bank early for the next tile's T1.
                        xts = xtspool.tile([128, 512], f32r)
                        nc.vector.tensor_copy(xts[:, 0:256], xtp[:, 0:256])
                        nc.scalar.copy(out=xts[:, 256:512], in_=xtp[:, 256:512])

                        # normf = ||f||^2 per token (squares read SBUF xts)
                        nf = nfpool.tile([128, 2], f32)
                        scr0 = scrpool.tile([128, 256], f32, tag="scr")
                        nc.scalar.activation(scr0[:], xts[:, 0:256], Act.Square,
                                             accum_out=nf[:, 0:1])
                        scr1 = scrpool.tile([128, 256], f32, tag="scr")
                        nc.scalar.activation(scr1[:], xts[:, 256:512], Act.Square,
                                             accum_out=nf[:, 1:2])

                        # M1: -2 f.e via fp16-split matmuls. Main term A
                        # (cols 0:32 / 64:96) accumulates fh*ph + fl*ph in
                        # psum; small correction B = fh*pl lands in the pl
                        # columns and is added afterwards on DVE.
                        sc = ps_sc.tile([128, 4 * K], f32)
                        nc.tensor.matmul(sc[:, 0:2 * K], fh0[:, s0], phpl0[:],
                                         start=True, stop=False)
                        nc.tensor.matmul(sc[:, 0:K], fl0[:, s0], ph0[:],
                                         start=False, stop=False)
                        nc.tensor.matmul(sc[:, 0:2 * K], fh1[:, s0], phpl1[:],
                                         start=False, stop=False)
                        nc.tensor.matmul(sc[:, 0:K], fl1[:, s0], ph1[:],
                                         start=False, stop=False)
                        nc.tensor.matmul(sc[:, 2 * K:4 * K], fh0[:, s1], phpl0[:],
                                         start=False, stop=False)
                        nc.tensor.matmul(sc[:, 2 * K:3 * K], fl0[:, s1], ph0[:],
                                         start=False, stop=False)
                        nc.tensor.matmul(sc[:, 2 * K:4 * K], fh1[:, s1], phpl1[:],
                                         start=False, stop=False)
                        nc.tensor.matmul(sc[:, 2 * K:3 * K], fl1[:, s1], ph1[:],
                                         start=False, stop=True)

                        # dist = ((A + ||f||^2) + B) + ||e||^2
                        # (two psum operands can't feed one DVE op, so A+nf
                        # lands in sbuf first, then B is added from psum)
                        dist = distpool.tile([128, 2 * K], f32)
                        tmp = distpool.tile([128, 2 * K], f32, tag="tmp")
                        nc.vector.tensor_scalar(tmp[:, 0:K], sc[:, 0:K],
                                                nf[:, 0:1], None, op0=Alu.add)
                        nc.vector.tensor_scalar(tmp[:, K:2 * K], sc[:, 2 * K:3 * K],
                                                nf[:, 1:2], None, op0=Alu.add)
                        tmp2 = distpool.tile([128, 2 * K], f32, tag="tmp2")
                        nc.vector.scalar_tensor_tensor(
                            out=tmp2[:, 0:K], in0=sc[:, K:2 * K], scalar=0.0,
                            in1=tmp[:, 0:K], op0=Alu.add, op1=Alu.add)
                        nc.vector.scalar_tensor_tensor(
                            out=tmp2[:, K:2 * K], in0=sc[:, 3 * K:4 * K], scalar=0.0,
                            in1=tmp[:, K:2 * K], op0=Alu.add, op1=Alu.add)
                        nc.vector.tensor_tensor(dist[:], tmp2[:], nek[:],
                                                op=Alu.add)

                        # row-min (loss partials) + one-hot
                        d3 = dist[:].rearrange("p (g k) -> p g k", k=K)
                        nc.vector.tensor_reduce(lossbuf[:, 2 * t:2 * t + 2],
                                                d3, axis=X, op=Alu.min)
                        # one-hot as f32r (exact 0/1; fast is_equal path,
                        # and a valid 1 cyc/row matmul operand for M3)
                        oht = ohtpool.tile([128, 2 * K], f32r)
                        nc.vector.tensor_scalar(oht[:, 0:K], dist[:, 0:K],
                                                lossbuf[:, 2 * t:2 * t + 1], None,
                                                op0=Alu.is_equal)
                        nc.vector.tensor_scalar(oht[:, K:2 * K], dist[:, K:2 * K],
                                                lossbuf[:, 2 * t + 1:2 * t + 2], None,
                                                op0=Alu.is_equal)

                        # T2: one-hot -> [k, token] (f32r transposes)
                        ohp = ps_oh.tile([K, 2 * 128], f32r)
                        nc.tensor.matmul(ohp[:, 0:128], oht[:, 0:K], id32r[:],
                                         is_transpose=True, start=True, stop=False)
                        nc.tensor.matmul(ohp[:, 128:256], oht[:, K:2 * K], id32r[:],
                                         is_transpose=True, start=False, stop=True)
                        ohs = ohspool.tile([K, 2 * 128], f16)
                        nc.scalar.copy(out=ohs[:], in_=ohp[:])

                        # M2: quantize = embT @ onehot (fp16)
                        q_ps = ps_q.tile([128, 512], f32)
                        nc.tensor.matmul(q_ps[:, 0:256], embt[:, 0:128], ohs[:],
                                         start=True, stop=False)
                        nc.tensor.matmul(q_ps[:, 256:512], embt[:, 128:256], ohs[:],
                                         start=False, stop=True)
                        nc.vector.tensor_copy(qb0[:, to:to + TT], q_ps[:, 0:256])
                        nc.vector.tensor_copy(qb1[:, to:to + TT], q_ps[:, 256:512])

                        # M3: embed_sumT [k, d] + counts col (persistent psum)
                        nc.tensor.matmul(es_ps[:, 0:D], oht[:, 0:K], xts[:, 0:256],
                                         start=first, stop=False)
                        nc.tensor.matmul(es_ps[:, D:D + 2], oht[:, 0:K], ones[:],
                                         start=False, stop=False)
                        nc.tensor.matmul(es_ps[:, 0:D], oht[:, K:2 * K], xts[:, 256:512],
                                         start=False, stop=False)
                        nc.tensor.matmul(es_ps[:, D:D + 2], oht[:, K:2 * K], ones[:],
                                         start=False, stop=last)
                        t += 1

                    nc.sync.dma_start(out=quant_d[b, 0:128, Q0:Q0 + QT], in_=qb0[:])
                    nc.sync.dma_start(out=quant_d[b, 128:256, Q0:Q0 + QT], in_=qb1[:])

            est_sb = accpool.tile([K, D + 2], f32)
            nc.vector.tensor_copy(est_sb[:], es_ps[:])
            nc.sync.dma_start(out=est_d[:], in_=est_sb[:])
            loss_sb = accpool.tile([128, 1], f32)
            nc.vector.tensor_reduce(loss_sb[:], lossbuf[:], axis=X, op=Alu.add)
            nc.sync.dma_start(out=loss_d[:], in_=loss_sb[:])

    nc.compile()
    return nc


def _get_program():
    if "nc" not in _cached:
        _cached["nc"] = _build_program()
    return _cached["nc"]


def make_in_maps(x, embed):
    p2en = (-2.0 * embed).astype(np.float32)
    ph = p2en.astype(np.float16)
    pl = (p2en - ph.astype(np.float32)).astype(np.float16)
    phpl = np.ascontiguousarray(np.concatenate([ph, pl], axis=1))
    nek1 = (embed * embed).sum(axis=0, dtype=np.float32)          # [K]
    nek = np.ascontiguousarray(
        np.broadcast_to(np.concatenate([nek1, nek1])[None, :], (128, 2 * K)),
        dtype=np.float32)
    embt = np.ascontiguousarray(embed.T).astype(np.float16)
    id32 = np.eye(128, dtype=np.float32)
    return [{
        "x": x[c * BPC:(c + 1) * BPC],
        "ph": ph, "phpl": phpl, "nek": nek, "embt": embt,
        "id32": id32, "id32r": id32,
        "ones": np.ones((128, 2), dtype=np.float32),
    } for c in range(NCORES)]


def kernel(x, embed, cluster_number, embed_avg, training):
    from concourse.bass_utils import run_bass_kernel_spmd

    x = np.ascontiguousarray(np.asarray(x, dtype=np.float32))
    embed = np.asarray(embed, dtype=np.float32)
    cluster_number = np.asarray(cluster_number, dtype=np.float32)
    embed_avg = np.asarray(embed_avg, dtype=np.float32)

    nc = _get_program()
    in_maps = make_in_maps(x, embed)
    res = run_bass_kernel_spmd(nc, in_maps, list(range(NCORES)))

    quant = np.empty((B, D, L), dtype=np.float32)
    counts = np.zeros(K, dtype=np.float32)
    embed_sum = np.zeros((D, K), dtype=np.float32)
    loss_sum = 0.0
    for c in range(NCORES):
        r = res.results[c]
        quant[c * BPC:(c + 1) * BPC] = r["quant"]
        est = r["est"]
        embed_sum += est[:, 0:D].T
        counts += est[:, D]
        loss_sum += float(r["lo"][:, 0].sum(dtype=np.float64))

    loss = np.float32(loss_sum / (B * L * D))

    train = bool(np.asarray(training).item()) if np.asarray(training).shape == () \
        else bool(training)
    if train:
        cn = (DECAY * cluster_number + (1.0 - DECAY) * counts).astype(np.float32)
        ea = (DECAY * embed_avg + (1.0 - DECAY) * embed_sum).astype(np.float32)
        n = cn.sum(dtype=np.float32)
        cnn = ((cn + EPS) / (n + K * EPS) * n).astype(np.float32)
        embed_out = (ea / cnn[None, :]).astype(np.float32)
    else:
        embed_out = embed

    return quant, embed_out, loss


# revision 18
# speedup vs baseline: 1.0608x; 1.0608x over previous
"""VQ codebook (DiscreteLayer) Trainium2 kernel.

kernel(**inputs) takes FULL inputs
  x [64, 256, 4096] f32, embed [256, 32] f32, cluster_number [32] f32,
  embed_avg [256, 32] f32, training scalar
and returns (quantize [64,256,4096] f32, embed_out [256,32] f32, loss f32),
matching the jax reference semantics.

Sharded data-parallel over batches across 8 NeuronCores (8 batches/core).

Per-core dataflow (measured-cost driven):
  - x streamed in quarter-batch tiles [128, 1024] (4KB-contiguous DMA runs),
    software-pipelined one quarter ahead
  - x split into exact fp16 hi/lo pairs (fh, fl): casts on ACT, residual
    subtracts on GPSIMD
  - scores -2*f.e from fp16 matmuls: main term A = fh*ph + fl*ph psum-
    accumulated; small correction B = fh*pl in separate psum columns.
    All products exact, fp32 accumulate -> fp32-matmul-class accuracy at
    1-cycle/row streaming and cheap single-pass LDWEIGHTS
  - x tile transposed on PE (fp32) -> token-major xts (fp16) for the
    embed_sum contraction; ||f||^2 per token via ACT Square+accum on the
    transposed psum
  - dist = ((A + ||f||^2) + B) + ||e||^2 in near-reference fp32 op order,
    argmin via reduce_min + is_equal one-hot (fp32; fp16-out is_equal
    hits a ~7x slower DVE path)
  - quantize = embT(fp16) @ onehot(fp16, via PE transpose) -- exact
    fp16-rounded embed rows; embed_sum/counts accumulate over the whole
    kernel in one PSUM bank with the one-hot stationary and token-major
    x (+ ones columns) streaming
Host: tiny EMA epilogue on [32]/[256,32] + loss scalar + gather.
"""

import numpy as np

B, D, L, K = 64, 256, 4096, 32
NCORES = 8
BPC = B // NCORES          # batches per core
TT = 256                   # tokens per tile
QT = 1024                  # tokens per quarter-batch DMA tile
TPQ = QT // TT             # tiles per quarter
NQ = L // QT               # quarters per batch
NTILES = BPC * (L // TT)   # 128 tiles per core
DECAY = 0.99
EPS = 1e-5

_cached = {}


def _build_program():
    import concourse.bacc as bacc
    import concourse.mybir as mybir
    from concourse.tile import TileContext

    f32 = mybir.dt.float32
    f16 = mybir.dt.float16
    Alu = mybir.AluOpType
    Act = mybir.ActivationFunctionType
    X = mybir.AxisListType.X

    nc = bacc.Bacc("TRN2", target_bir_lowering=False, debug=False,
                   num_devices=NCORES)

    xd = nc.dram_tensor("x", [BPC, D, L], f32, kind="ExternalInput").ap()
    ph_d = nc.dram_tensor("ph", [D, K], f16, kind="ExternalInput").ap()
    phpl_d = nc.dram_tensor("phpl", [D, 2 * K], f16, kind="ExternalInput").ap()
    nek_d = nc.dram_tensor("nek", [128, 2 * K], f32, kind="ExternalInput").ap()
    embt_d = nc.dram_tensor("embt", [K, D], f16, kind="ExternalInput").ap()
    id32_d = nc.dram_tensor("id32", [128, 128], f32, kind="ExternalInput").ap()

    quant_d = nc.dram_tensor("quant", [BPC, D, L], f32, kind="ExternalOutput").ap()
    est_d = nc.dram_tensor("est", [K, D + 1], f32, kind="ExternalOutput").ap()
    loss_d = nc.dram_tensor("lo", [128, 1], f32, kind="ExternalOutput").ap()

    with TileContext(nc) as tc:
        with tc.tile_pool(name="const", bufs=1) as constp, \
             tc.tile_pool(name="xin", bufs=4) as xpool, \
             tc.tile_pool(name="fhl", bufs=4) as fpool, \
             tc.tile_pool(name="qb", bufs=4) as qbpool, \
             tc.tile_pool(name="xts", bufs=3) as xtspool, \
             tc.tile_pool(name="dist", bufs=3) as distpool, \
             tc.tile_pool(name="oht", bufs=3) as ohtpool, \
             tc.tile_pool(name="ohs", bufs=3) as ohspool, \
             tc.tile_pool(name="nf", bufs=4) as nfpool, \
             tc.tile_pool(name="scr", bufs=2) as scrpool, \
             tc.tile_pool(name="acc", bufs=1) as accpool, \
             tc.tile_pool(name="ps_xt", bufs=2, space="PSUM") as ps_xt, \
             tc.tile_pool(name="ps_sc", bufs=2, space="PSUM") as ps_sc, \
             tc.tile_pool(name="ps_oh", bufs=1, space="PSUM") as ps_oh, \
             tc.tile_pool(name="ps_q", bufs=2, space="PSUM") as ps_q, \
             tc.tile_pool(name="ps_es", bufs=1, space="PSUM") as ps_es:

            ph0 = constp.tile([128, K], f16)
            nc.sync.dma_start(out=ph0[:], in_=ph_d[0:128, :])
            ph1 = constp.tile([128, K], f16)
            nc.sync.dma_start(out=ph1[:], in_=ph_d[128:256, :])
            phpl0 = constp.tile([128, 2 * K], f16)
            nc.sync.dma_start(out=phpl0[:], in_=phpl_d[0:128, :])
            phpl1 = constp.tile([128, 2 * K], f16)
            nc.sync.dma_start(out=phpl1[:], in_=phpl_d[128:256, :])
            nek = constp.tile([128, 2 * K], f32)
            nc.sync.dma_start(out=nek[:], in_=nek_d[:])
            embt = constp.tile([K, D], f16)
            nc.sync.dma_start(out=embt[:], in_=embt_d[:])
            id32 = constp.tile([128, 128], f32)
            nc.sync.dma_start(out=id32[:], in_=id32_d[:])

            lossbuf = accpool.tile([128, 2 * NTILES], f32)
            es_ps = ps_es.tile([K, D + 1], f32)

            def prep_quarter(b, qi):
                """DMA in one quarter and build its fp16 hi/lo split."""
                Q0 = qi * QT
                x0q = xpool.tile([128, QT], f32, tag="x")
                nc.sync.dma_start(out=x0q[:], in_=xd[b, 0:128, Q0:Q0 + QT])
                x1q = xpool.tile([128, QT], f32, tag="x")
                nc.sync.dma_start(out=x1q[:], in_=xd[b, 128:256, Q0:Q0 + QT])
                # casts on ACT, subtracts on GPSIMD (idle; DVE/ACT are busy)
                fh0 = fpool.tile([128, QT], f16, tag="fh")
                nc.scalar.copy(out=fh0[:], in_=x0q[:])
                fl0 = fpool.tile([128, QT], f16, tag="fl")
                nc.gpsimd.tensor_tensor(fl0[:], x0q[:], fh0[:], op=Alu.subtract)
                fh1 = fpool.tile([128, QT], f16, tag="fh")
                nc.scalar.copy(out=fh1[:], in_=x1q[:])
                fl1 = fpool.tile([128, QT], f16, tag="fl")
                nc.gpsimd.tensor_tensor(fl1[:], x1q[:], fh1[:], op=Alu.subtract)
                return x0q, x1q, fh0, fl0, fh1, fl1

            quarters = [(b, qi) for b in range(BPC) for qi in range(NQ)]
            pending = prep_quarter(*quarters[0])
            t = 0
            for bq in range(len(quarters)):
                b, qi = quarters[bq]
                Q0 = qi * QT
                x0q, x1q, fh0, fl0, fh1, fl1 = pending
                if bq + 1 < len(quarters):
                    pending = prep_quarter(*quarters[bq + 1])

                qb0 = qbpool.tile([128, QT], f32, tag="qb")
                qb1 = qbpool.tile([128, QT], f32, tag="qb")

                for i in range(TPQ):
                    to = i * TT          # token offset within quarter
                    first, last = (t == 0), (t == NTILES - 1)
                    s0, s1 = slice(to, to + 128), slice(to + 128, to + 256)

                    # T1: transpose x tile -> token-major (one psum bank)
                    xtp = ps_xt.tile([128, 512], f32)
                    nc.tensor.matmul(xtp[:, 0:128], x0q[:, s0], id32[:],
                                     is_transpose=True, start=True, stop=False)
                    nc.tensor.matmul(xtp[:, 128:256], x1q[:, s0], id32[:],
                                     is_transpose=True, start=False, stop=False)
                    nc.tensor.matmul(xtp[:, 256:384], x0q[:, s1], id32[:],
                                     is_transpose=True, start=False, stop=False)
                    nc.tensor.matmul(xtp[:, 384:512], x1q[:, s1], id32[:],
                                     is_transpose=True, start=False, stop=True)

                    # normf = ||f||^2 per token (ACT Square + row accumulate)
                    nf = nfpool.tile([128, 2], f32)
                    scr0 = scrpool.tile([128, 256], f32, tag="scr")
                    nc.scalar.activation(scr0[:], xtp[:, 0:256], Act.Square,
                                         accum_out=nf[:, 0:1])
                    scr1 = scrpool.tile([128, 256], f32, tag="scr")
                    nc.scalar.activation(scr1[:], xtp[:, 256:512], Act.Square,
                                         accum_out=nf[:, 1:2])

                    # xts: token-major x in fp16 (+ ones cols for counts)
                    xts = xtspool.tile([128, 514], f16)
                    nc.vector.tensor_copy(xts[:, 0:256], xtp[:, 0:256])
                    nc.scalar.copy(out=xts[:, 257:513], in_=xtp[:, 256:512])
                    nc.gpsimd.memset(xts[:, 256:257], 1.0)
                    nc.gpsimd.memset(xts[:, 513:514], 1.0)

                    # M1: -2 f.e via fp16-split matmuls. Main term A
                    # (cols 0:32 / 64:96) accumulates fh*ph + fl*ph in
                    # psum; small correction B = fh*pl lands in the pl
                    # columns and is added afterwards on DVE.
                    sc = ps_sc.tile([128, 4 * K], f32)
                    nc.tensor.matmul(sc[:, 0:2 * K], fh0[:, s0], phpl0[:],
                                     start=True, stop=False)
                    nc.tensor.matmul(sc[:, 0:K], fl0[:, s0], ph0[:],
                                     start=False, stop=False)
                    nc.tensor.matmul(sc[:, 0:2 * K], fh1[:, s0], phpl1[:],
                                     start=False, stop=False)
                    nc.tensor.matmul(sc[:, 0:K], fl1[:, s0], ph1[:],
                                     start=False, stop=False)
                    nc.tensor.matmul(sc[:, 2 * K:4 * K], fh0[:, s1], phpl0[:],
                                     start=False, stop=False)
                    nc.tensor.matmul(sc[:, 2 * K:3 * K], fl0[:, s1], ph0[:],
                                     start=False, stop=False)
                    nc.tensor.matmul(sc[:, 2 * K:4 * K], fh1[:, s1], phpl1[:],
                                     start=False, stop=False)
                    nc.tensor.matmul(sc[:, 2 * K:3 * K], fl1[:, s1], ph1[:],
                                     start=False, stop=True)

                    # dist = ((A + ||f||^2) + B) + ||e||^2
                    # (two psum operands can't feed one DVE op, so A+nf
                    # lands in sbuf first, then B is added from psum)
                    dist = distpool.tile([128, 2 * K], f32)
                    tmp = distpool.tile([128, 2 * K], f32, tag="tmp")
                    nc.vector.tensor_scalar(tmp[:, 0:K], sc[:, 0:K],
                                            nf[:, 0:1], None, op0=Alu.add)
                    nc.vector.tensor_scalar(tmp[:, K:2 * K], sc[:, 2 * K:3 * K],
                                            nf[:, 1:2], None, op0=Alu.add)
                    tmp2 = distpool.tile([128, 2 * K], f32, tag="tmp2")
                    nc.vector.scalar_tensor_tensor(
                        out=tmp2[:, 0:K], in0=sc[:, K:2 * K], scalar=0.0,
                        in1=tmp[:, 0:K], op0=Alu.add, op1=Alu.add)
                    nc.vector.scalar_tensor_tensor(
                        out=tmp2[:, K:2 * K], in0=sc[:, 3 * K:4 * K], scalar=0.0,
                        in1=tmp[:, K:2 * K], op0=Alu.add, op1=Alu.add)
                    nc.vector.tensor_tensor(dist[:], tmp2[:], nek[:],
                                            op=Alu.add)

                    # row-min (loss partials) + one-hot
                    d3 = dist[:].rearrange("p (g k) -> p g k", k=K)
                    nc.vector.tensor_reduce(lossbuf[:, 2 * t:2 * t + 2],
                                            d3, axis=X, op=Alu.min)
                    # one-hot in fp32 (fp16-out is_equal hits a slow
                    # DVE path: ~1.7us vs 240ns); fp16 copy for M3 on GP
                    oht = ohtpool.tile([128, 2 * K], f32)
                    nc.vector.tensor_scalar(oht[:, 0:K], dist[:, 0:K],
                                            lossbuf[:, 2 * t:2 * t + 1], None,
                                            op0=Alu.is_equal)
                    nc.vector.tensor_scalar(oht[:, K:2 * K], dist[:, K:2 * K],
                                            lossbuf[:, 2 * t + 1:2 * t + 2], None,
                                            op0=Alu.is_equal)
                    oht16 = ohtpool.tile([128, 2 * K], f16, tag="oht16")
                    nc.gpsimd.tensor_copy(oht16[:], oht[:])

                    # T2: one-hot -> [k, token] (fp32 transposes)
                    ohp = ps_oh.tile([K, 2 * 128], f32)
                    nc.tensor.matmul(ohp[:, 0:128], oht[:, 0:K], id32[:],
                                     is_transpose=True, start=True, stop=False)
                    nc.tensor.matmul(ohp[:, 128:256], oht[:, K:2 * K], id32[:],
                                     is_transpose=True, start=False, stop=True)
                    ohs = ohspool.tile([K, 2 * 128], f16)
                    nc.scalar.copy(out=ohs[:], in_=ohp[:])

                    # M2: quantize = embT @ onehot (fp16)
                    q_ps = ps_q.tile([128, 512], f32)
                    nc.tensor.matmul(q_ps[:, 0:256], embt[:, 0:128], ohs[:],
                                     start=True, stop=False)
                    nc.tensor.matmul(q_ps[:, 256:512], embt[:, 128:256], ohs[:],
                                     start=False, stop=True)
                    nc.vector.tensor_copy(qb0[:, to:to + TT], q_ps[:, 0:256])
                    nc.vector.tensor_copy(qb1[:, to:to + TT], q_ps[:, 256:512])

                    # M3: embed_sumT [k, d] + counts col (persistent psum)
                    nc.tensor.matmul(es_ps[:], oht16[:, 0:K], xts[:, 0:257],
                                     start=first, stop=False)
                    nc.tensor.matmul(es_ps[:], oht16[:, K:2 * K], xts[:, 257:514],
                                     start=False, stop=last)
                    t += 1

                nc.sync.dma_start(out=quant_d[b, 0:128, Q0:Q0 + QT], in_=qb0[:])
                nc.sync.dma_start(out=quant_d[b, 128:256, Q0:Q0 + QT], in_=qb1[:])

            est_sb = accpool.tile([K, D + 1], f32)
            nc.vector.tensor_copy(est_sb[:], es_ps[:])
            nc.sync.dma_start(out=est_d[:], in_=est_sb[:])
            loss_sb = accpool.tile([128, 1], f32)
            nc.vector.tensor_reduce(loss_sb[:], lossbuf[:], axis=X, op=Alu.add)
            nc.sync.dma_start(out=loss_d[:], in_=loss_sb[:])

    nc.compile()
    return nc


def _get_program():
    if "nc" not in _cached:
        _cached["nc"] = _build_program()
    return _cached["nc"]


def make_in_maps(x, embed):
    p2en = (-2.0 * embed).astype(np.float32)
    ph = p2en.astype(np.float16)
    pl = (p2en - ph.astype(np.float32)).astype(np.float16)
    phpl = np.ascontiguousarray(np.concatenate([ph, pl], axis=1))
    nek1 = (embed * embed).sum(axis=0, dtype=np.float32)          # [K]
    nek = np.ascontiguousarray(
        np.broadcast_to(np.concatenate([nek1, nek1])[None, :], (128, 2 * K)),
        dtype=np.float32)
    embt = np.ascontiguousarray(embed.T).astype(np.float16)
    id32 = np.eye(128, dtype=np.float32)
    return [{
        "x": x[c * BPC:(c + 1) * BPC],
        "ph": ph, "phpl": phpl, "nek": nek, "embt": embt,
        "id32": id32,
    } for c in range(NCORES)]


def kernel(x, embed, cluster_number, embed_avg, training):
    from concourse.bass_utils import run_bass_kernel_spmd

    x = np.ascontiguousarray(np.asarray(x, dtype=np.float32))
    embed = np.asarray(embed, dtype=np.float32)
    cluster_number = np.asarray(cluster_number, dtype=np.float32)
    embed_avg = np.asarray(embed_avg, dtype=np.float32)

    nc = _get_program()
    in_maps = make_in_maps(x, embed)
    res = run_bass_kernel_spmd(nc, in_maps, list(range(NCORES)))

    quant = np.empty((B, D, L), dtype=np.float32)
    counts = np.zeros(K, dtype=np.float32)
    embed_sum = np.zeros((D, K), dtype=np.float32)
    loss_sum = 0.0
    for c in range(NCORES):
        r = res.results[c]
        quant[c * BPC:(c + 1) * BPC] = r["quant"]
        est = r["est"]
        embed_sum += est[:, 0:D].T
        counts += est[:, D]
        loss_sum += float(r["lo"][:, 0].sum(dtype=np.float64))

    loss = np.float32(loss_sum / (B * L * D))

    train = bool(np.asarray(training).item()) if np.asarray(training).shape == () \
        else bool(training)
    if train:
        cn = (DECAY * cluster_number + (1.0 - DECAY) * counts).astype(np.float32)
        ea = (DECAY * embed_avg + (1.0 - DECAY) * embed_sum).astype(np.float32)
        n = cn.sum(dtype=np.float32)
        cnn = ((cn + EPS) / (n + K * EPS) * n).astype(np.float32)
        embed_out = (ea / cnn[None, :]).astype(np.float32)
    else:
        embed_out = embed

    return quant, embed_out, loss
